# revision 32
# baseline (speedup 1.0000x reference)
"""GQA attention (B=2, S=2048, H=16, HKV=8, D=128) + RoPE + QKV/O proj
on 8 TRN2 NeuronCores.

Sharding: tensor-parallel by head. Core c computes QKV projection for its
2 q-heads / 1 kv-head over all tokens, RoPE, and full (non-causal)
attention for those heads.  An on-chip AllToAll then redistributes the
per-head attention outputs so core c holds *all* heads for its 512-token
block, and each core computes the output projection for its token block.
Host-side work is shard/concat plus bf16 quantization of the matmul
operands (the compute-precision choice for the PE; fp32 PSUM
accumulation throughout, fp32 softmax denominator).

All matmuls run with bf16 operands (TRN2 PE streams bf16 at 1
cycle/row vs 2 for fp32/f32r, and bf16 LDWEIGHTS is fast enough to
hide under the matmuls).  RoPE and softmax run in fp32 on the DVE/ACT
engines.  Softmax without max-subtraction (scores are in [-9, 9] for
this problem's data distribution).
"""
import sys
import types

import numpy as np
import ml_dtypes

BFNP = ml_dtypes.bfloat16


def _install_ntff_hook():
    """The container's antenv stub lacks axon_hooks; shim it so
    run_bass_kernel_spmd(trace=True) can capture NTFF profiles."""
    try:
        import antenv.axon_hooks  # noqa: F401
        return
    except ImportError:
        pass
    try:
        import trn_agent_boot.trn_boot as tb
        hook = tb._ntff_profile_via_ctypes("/opt/axon/libaxon_pjrt.so")
        mod = types.ModuleType("antenv.axon_hooks")
        mod.get_axon_ntff_profile_hook = lambda: hook
        sys.modules["antenv.axon_hooks"] = mod
    except Exception:
        pass


_install_ntff_hook()

import concourse.mybir as mybir  # noqa: E402
import concourse.tile as tile  # noqa: E402
from concourse import bacc  # noqa: E402
from concourse.bass_utils import run_bass_kernel_spmd  # noqa: E402

F32 = mybir.dt.float32
F32R = mybir.dt.float32r
BF16 = mybir.dt.bfloat16
AF = mybir.ActivationFunctionType

B, S, HID = 2, 2048, 2048
H, HKV, D = 16, 8, 128
NCORES = 8
TOK = B * S              # 4096 stacked tokens (batch-major)
TPC = TOK // NCORES      # 512 tokens owned per core
NTB = TOK // 512         # 8 phase-1 token blocks
KCH = HID // 128         # 16 contraction chunks
SCALE = 1.0 / float(np.sqrt(D))
HPC = H // NCORES        # 2 q-heads per core


def build():
    nc = bacc.Bacc("TRN2", target_bir_lowering=False, debug=False,
                   num_devices=NCORES)

    hT = nc.dram_tensor("hT", [HID, TOK], BF16, kind="ExternalInput")
    wqT = nc.dram_tensor("wqT", [HID, 4 * D], BF16, kind="ExternalInput")
    woT = nc.dram_tensor("woT", [H * D, HID], BF16, kind="ExternalInput")
    cosT = nc.dram_tensor("cosT", [D, TOK], F32, kind="ExternalInput")
    sinST = nc.dram_tensor("sinST", [D, TOK], F32, kind="ExternalInput")
    ones_col = nc.dram_tensor("ones_col", [128, 1], BF16, kind="ExternalInput")
    ones_row = nc.dram_tensor("ones_row", [1, 128], BF16, kind="ExternalInput")
    ident = nc.dram_tensor("ident", [128, 128], F32, kind="ExternalInput")
    out = nc.dram_tensor("out", [TPC, HID], F32, kind="ExternalOutput")

    hT_v = hT[:].rearrange("(c p) t -> p c t", p=128)      # [128, 16, 4096]
    wqT_v = wqT[:].rearrange("(c p) m -> p c m", p=128)    # [128, 16, 512]
    woT_v = woT[:].rearrange("(c p) o -> p c o", p=128)    # [128, 16, 2048]

    with tile.TileContext(nc) as tc:
        with (
            tc.tile_pool(name="cst", bufs=1) as cst,
            tc.tile_pool(name="res", bufs=1) as res,
            tc.tile_pool(name="med", bufs=16) as med,
            tc.tile_pool(name="tbl", bufs=2) as tblp,
            tc.tile_pool(name="wk", bufs=8) as wkp,
            tc.tile_pool(name="pp", bufs=3) as ppp,
            tc.tile_pool(name="rr", bufs=2) as rrp,
            tc.tile_pool(name="qs", bufs=3) as qsp,
            tc.tile_pool(name="aa", bufs=1) as aap,
            tc.tile_pool(name="wop", bufs=32) as wop,
            tc.tile_pool(name="psS", bufs=2, space="PSUM") as psS,
            tc.tile_pool(name="psacc", bufs=3, space="PSUM") as psacc,
            tc.tile_pool(name="psden", bufs=1, space="PSUM") as psden,
            tc.tile_pool(name="dram", bufs=1, space="DRAM") as dram,
        ):
            # ---- constants ----
            ident_s = cst.tile([128, 128], F32, name="ident_s")
            ident_b = cst.tile([128, 128], BF16, name="ident_b")
            ones_c = cst.tile([128, 1], BF16, name="ones_c")
            ones_r = cst.tile([1, 128], BF16, name="ones_r")
            nc.sync.dma_start(ident_s[:], ident[:])
            nc.sync.dma_start(ones_c[:], ones_col[:])
            nc.sync.dma_start(ones_r[:], ones_row[:])
            with nc.allow_low_precision(reason="bf16 identity"):
                nc.scalar.copy(ident_b[:], ident_s[:])

            # ---- resident tensors (bf16) ----
            wq_s = res.tile([128, KCH, 4 * D], BF16, name="wq_s")  # 16KB/p
            kT = res.tile([128, TOK], BF16, name="kT")             # 8KB/p
            V = res.tile([128, 32, 128], BF16, name="V")           # 8KB/p

            # DRAM bounces (bf16)
            qT_d = dram.tile([HPC, 128, TOK], BF16, name="qT_d")
            a2a_in = [
                dram.tile([NCORES, 128, TPC], BF16, name=f"a2a_in{h}")
                for h in range(HPC)
            ]
            a2a_out = [
                dram.tile([NCORES, 128, TPC], BF16, name=f"a2a_out{h}")
                for h in range(HPC)
            ]

            # ========== Phase 1 (QKV + RoPE) / Phase 2 (attention) ==========
            # Emitted batch-interleaved: tb 0-3 (batch 0) -> h0/b0 attention
            # -> tb 4-7 -> h0/b1 attention -> A2A0 -> h1 attention -> A2A1,
            # so the scheduler can fill phase-1 DMA stalls with attention
            # matmuls and vice versa.
            def ph1_block(tb, hook_a=None, hook_b=None):
                t0 = tb * 512
                hid_t = []
                if tb == 0:
                    # chunked loads on the (idle) scalar queue so the first
                    # matmuls only wait for their own contraction chunk
                    for hq in range(8):
                        nc.scalar.dma_start(wq_s[:, 2 * hq:2 * hq + 2, :],
                                            wqT_v[:, 2 * hq:2 * hq + 2, :])
                for hq in range(8):
                    ht_ = med.tile([128, 2, 512], BF16, tag="med",
                                   name=f"hid{hq}")
                    # tb1 splits across sync+scalar: both queues feed the
                    # HBM-bound startup window in parallel
                    eng = nc.scalar if (tb == 1 and hq % 2 == 1) else nc.sync
                    eng.dma_start(
                        ht_[:], hT_v[:, 2 * hq:2 * hq + 2, t0:t0 + 512])
                    hid_t.append(ht_)
                cosc = tblp.tile([128, 512], F32, tag="cosc", name="cosc")
                sinc = tblp.tile([128, 512], F32, tag="sinc", name="sinc")
                nc.sync.dma_start(cosc[:], cosT[:, t0:t0 + 512])
                nc.sync.dma_start(sinc[:], sinST[:, t0:t0 + 512])

                for m in range(4):  # q0, q1, k, v
                    ps = psacc.tile([128, 512], F32, tag="ps_a",
                                    name="ps_qkv")
                    for kk in range(KCH):
                        nc.tensor.matmul(
                            ps[:],
                            wq_s[:, kk, m * 128:(m + 1) * 128],
                            hid_t[kk // 2][:, kk % 2, :],
                            start=(kk == 0), stop=(kk == KCH - 1),
                        )
                    if m == 0 and hook_a is not None:
                        hook_a()
                    if m == 1 and hook_b is not None:
                        hook_b()
                    if m == 3:
                        # V: evict (on gpsimd - the ACT queue is exp-bound)
                        # then transpose to [t, d] layout
                        vt = wkp.tile([128, 512], BF16, tag="wkb", name="vt")
                        with nc.allow_low_precision(reason="bf16 V"):
                            nc.vector.tensor_copy(vt[:], ps[:])
                        for i in range(4):
                            trp = psden.tile([128, 128], BF16, tag="ps_d",
                                             name="ps_tr")
                            nc.tensor.matmul(
                                trp[:], vt[:, i * 128:(i + 1) * 128],
                                ident_b[:], is_transpose=True,
                            )
                            with nc.allow_low_precision(reason="bf16 V"):
                                nc.scalar.copy(V[:, tb * 4 + i, :], trp[:])
                    else:
                        # q/k: RoPE.  dest = ps*cos + swap(ps)*sinS
                        qmul = wkp.tile([128, 512], F32, tag="wk",
                                        name="qmul")
                        qraw = wkp.tile([128, 512], F32, tag="wk", name="qraw")
                        rot = wkp.tile([128, 512], F32, tag="wk", name="rot")
                        nc.scalar.copy(qraw[:], ps[:])
                        nc.vector.tensor_mul(qmul[:], ps[:], cosc[:])
                        nc.gpsimd.dma_start(rot[0:64, :], qraw[64:128, :])
                        nc.gpsimd.dma_start(rot[64:128, :], qraw[0:64, :])
                        nc.vector.tensor_mul(rot[:], rot[:], sinc[:])
                        if m == 2:
                            dest = kT[:, t0:t0 + 512]
                        else:
                            dest = wkp.tile([128, 512], BF16, tag="wkb",
                                            name="qdest")
                        with nc.allow_low_precision(reason="bf16 q/k"):
                            nc.vector.tensor_add(dest[:], qmul[:], rot[:])
                        if m != 2:
                            # gpsimd queue: keeps this RoPE-gated write out
                            # of the sync queue's DMA-semaphore rotation,
                            # which would stall later hid loads behind it
                            nc.gpsimd.dma_start(qT_d[m, :, t0:t0 + 512],
                                                dest[:])

            # The softmax denominator chain (fold -> ones-matmul -> DVE
            # reciprocal -> rank-1 broadcast matmul) is a cross-engine
            # latency chain.  It is split off the main attention loop and
            # deferred into the *next* group's matmul stream so the two PE
            # matmuls embedded in it (psD, psB) never stall the strict-FIFO
            # PE queue: their inputs are ready by the time the PE drains
            # the next group's score/AV matmuls down to them.
            def ph2_main(h, b, qb, hook_a=None, hook_b=None):
                q0 = b * S + qb * 512
                qsl = qsp.tile([128, 512], BF16, tag="qs", name="qsl")
                nc.sync.dma_start(qsl[:], qT_d[h, :, q0:q0 + 512])
                psA = psacc.tile([128, 512], F32, tag="ps_a", name="ps_av")
                R = rrp.tile([128, 1024], F32, tag="rr", name="R")
                Pprev = None
                for i in range(8):  # pairs of 128-token k/v blocks
                    ta = b * S + (2 * i) * 128
                    tbk = b * S + (2 * i + 1) * 128
                    psSt = psS.tile([128, 1024], F32, tag="ps_s",
                                    name="ps_sc")
                    nc.tensor.matmul(psSt[:, 0:512],
                                     kT[:, ta:ta + 128], qsl[:],
                                     start=True, stop=True)
                    nc.tensor.matmul(psSt[:, 512:1024],
                                     kT[:, tbk:tbk + 128], qsl[:],
                                     start=True, stop=True)
                    P = ppp.tile([128, 1024], BF16, tag="pp", name="P")
                    with nc.allow_low_precision(reason="bf16 attn probs"):
                        nc.scalar.activation(P[:], psSt[:], AF.Exp,
                                             scale=SCALE)
                    nc.tensor.matmul(psA[:], V[:, b * 16 + 2 * i, :],
                                     P[:, 0:512],
                                     start=(i == 0), stop=False)
                    nc.tensor.matmul(psA[:], V[:, b * 16 + 2 * i + 1, :],
                                     P[:, 512:1024],
                                     start=False, stop=(i == 7))
                    if i == 1:
                        nc.vector.tensor_add(R[:], Pprev[:], P[:])
                    elif i >= 2:
                        nc.vector.tensor_add(R[:], R[:], P[:])
                    Pprev = P
                    if i == 2 and hook_a is not None:
                        hook_a()
                    if i == 5 and hook_b is not None:
                        hook_b()
                return {"h": h, "b": b, "qb": qb, "psA": psA, "R": R}

            def ph2_tail_a(ctx):
                R = ctx["R"]
                R2 = wkp.tile([128, 512], BF16, tag="wkb", name="R2")
                with nc.allow_low_precision(reason="bf16 softmax denom"):
                    nc.vector.tensor_add(R2[:], R[:, 0:512], R[:, 512:1024])
                psD = psden.tile([1, 512], F32, tag="ps_d", name="ps_den")
                nc.tensor.matmul(psD[:], ones_c[:], R2[:],
                                 start=True, stop=True)
                rec32 = wkp.tile([1, 512], F32, tag="wk", name="rec32")
                nc.vector.reciprocal_approx_fast(rec32[:], psD[:])
                rec = wkp.tile([1, 512], BF16, tag="wkb", name="rec")
                with nc.allow_low_precision(
                        reason="bf16 rounding of softmax denom"):
                    nc.vector.tensor_copy(rec[:], rec32[:])
                ctx["rec"] = rec

            def ph2_tail_b(ctx):
                psB = psden.tile([128, 512], F32, tag="ps_d", name="ps_bc")
                nc.tensor.matmul(psB[:], ones_r[:], ctx["rec"][:],
                                 start=True, stop=True)
                bcB = wkp.tile([128, 512], F32, tag="wk", name="bcB")
                nc.vector.tensor_copy(bcB[:], psB[:])
                attn = wkp.tile([128, 512], BF16, tag="wkb", name="attn")
                with nc.allow_low_precision(reason="bf16 attention values"):
                    nc.vector.tensor_mul(attn[:], ctx["psA"][:], bcB[:])
                nc.sync.dma_start(a2a_in[ctx["h"]][4 * ctx["b"] + ctx["qb"]],
                                  attn[:])

            pending = [None]

            def _hook_a():
                if pending[0] is not None:
                    ph2_tail_a(pending[0])

            def _hook_b():
                if pending[0] is not None:
                    ph2_tail_b(pending[0])
                    pending[0] = None

            def ph2_group(h, b, qb):
                ctx = ph2_main(h, b, qb, hook_a=_hook_a, hook_b=_hook_b)
                pending[0] = ctx

            def ph2_flush():
                if pending[0] is not None:
                    ph2_tail_a(pending[0])
                    ph2_tail_b(pending[0])
                    pending[0] = None

            # ================= Phase 3: output projection =================
            # wo tiles live in their own pool (no ring-slot dependency on
            # the hid tiles) and load via the gpsimd queue so they are not
            # stuck behind phase-2 traffic on the sync engine's DMA queue.
            def wo_load(n):
                n0 = n * 512
                tiles = []
                for wq4 in range(8):
                    wt_ = wop.tile([128, 2, 512], BF16, tag="wop",
                                   name=f"wo{wq4}")
                    nc.gpsimd.dma_start(
                        wt_[:], woT_v[:, 2 * wq4:2 * wq4 + 2, n0:n0 + 512])
                    tiles.append(wt_)
                return tiles

            def half_mms(psO, att, kh0, wo_t, m, first, close):
                for j in range(8):
                    kh = 2 * j + kh0
                    nc.tensor.matmul(
                        psO[:],
                        att[:, j, m * 128:(m + 1) * 128],
                        wo_t[kh // 2][:, kh % 2, :],
                        start=(first and j == 0), stop=(close and j == 7),
                    )

            def finish(psO, m, n, ev=None):
                oev = wkp.tile([128, 512], F32, tag="wk", name="oev")
                if ev is None:
                    nc.scalar.copy(oev[:], psO[:])
                else:
                    nc.vector.tensor_add(oev[:], psO[:], ev[:])
                nc.sync.dma_start(out[m * 128:(m + 1) * 128,
                                      n * 512:(n + 1) * 512], oev[:])

            # ---------------- emission schedule ----------------
            # tb 4-7 are woven between the first attention groups: the PE
            # queue is strict FIFO, so a ph2 group whose inputs are still
            # settling must have ready QKV matmuls emitted *before* it,
            # not after.
            for tb in range(4):
                ph1_block(tb)
            ph1_block(4)
            ph2_group(0, 0, 0)
            ph1_block(5, _hook_a, _hook_b)
            ph2_group(0, 0, 1)
            ph1_block(6, _hook_a, _hook_b)
            ph2_group(0, 0, 2)
            ph1_block(7, _hook_a, _hook_b)
            ph2_group(0, 0, 3)
            for qb in range(4):
                ph2_group(0, 1, qb)
            ph2_flush()
            nc.gpsimd.collective_compute(
                "AllToAll", mybir.AluOpType.bypass,
                replica_groups=[list(range(NCORES))],
                ins=[a2a_in[0].opt()], outs=[a2a_out[0].opt()],
            )
            # o-proj weight prefetch sits on the gpsimd queue behind the
            # A2A0 trigger: no HBM contention with phase-1 loads, done
            # well before phase 3 needs them.
            wo_t = [wo_load(n) for n in range(4)]
            att_ev = aap.tile([128, 8, 512], BF16, name="att_ev")
            for j in range(NCORES):
                nc.gpsimd.dma_start(att_ev[:, j, :], a2a_out[0][j])
            for b in range(B):
                for qb in range(4):
                    ph2_group(1, b, qb)
            ph2_flush()
            nc.gpsimd.collective_compute(
                "AllToAll", mybir.AluOpType.bypass,
                replica_groups=[list(range(NCORES))],
                ins=[a2a_in[1].opt()], outs=[a2a_out[1].opt()],
            )
            # While the AllToAll is in flight the PE runs all the even-head
            # (att_ev) halves: n=0,1 as closed partial groups that are
            # evicted to SBUF (bf16), freeing the banks for n=2,3 whose
            # groups stay open across the collective.
            def even_quad(wo_n, close):
                tiles = []
                for m in range(4):
                    pool, tag = ((psacc, "ps_a") if m < 3 else
                                 (psden, "ps_d"))
                    psO = pool.tile([128, 512], F32, tag=tag, name="ps_o")
                    half_mms(psO, att_ev, 0, wo_n, m, first=True,
                             close=close)
                    tiles.append(psO)
                return tiles

            def even_pair(wo_n, close):
                tiles = []
                for mp in range(2):
                    psO2 = psS.tile([128, 2, 512], F32, tag="ps_s",
                                    name="ps_o2")
                    for mh in range(2):
                        half_mms(psO2[:, mh, :], att_ev, 0, wo_n,
                                 2 * mp + mh, first=True, close=close)
                    tiles.append(psO2)
                return tiles

            psO_n0 = even_quad(wo_t[0], close=True)
            ev0 = []
            for m in range(4):
                ev = wkp.tile([128, 512], BF16, tag="ev", name="ev0")
                with nc.allow_low_precision(reason="bf16 o-proj partial"):
                    nc.scalar.copy(ev[:], psO_n0[m][:])
                ev0.append(ev)
            psO_n1 = even_pair(wo_t[1], close=True)
            ev1 = []
            for mp in range(2):
                for mh in range(2):
                    ev = wkp.tile([128, 512], BF16, tag="ev", name="ev1")
                    with nc.allow_low_precision(reason="bf16 o-proj partial"):
                        nc.scalar.copy(ev[:], psO_n1[mp][:, mh, :])
                    ev1.append(ev)
            psO_n2 = even_quad(wo_t[2], close=False)
            psO_n3 = even_pair(wo_t[3], close=False)
            att_od = aap.tile([128, 8, 512], BF16, name="att_od")
            for j in range(NCORES):
                nc.sync.dma_start(att_od[:, j, :], a2a_out[1][j])
            # odd-head halves: n=2,3 accumulate into the open groups;
            # n=0,1 restart in the freed banks and add back the evicted
            # even partials during the final copy.
            for m in range(4):
                half_mms(psO_n2[m], att_od, 1, wo_t[2], m, first=False,
                         close=True)
                finish(psO_n2[m], m, 2)
            for mp in range(2):
                for mh in range(2):
                    m = 2 * mp + mh
                    half_mms(psO_n3[mp][:, mh, :], att_od, 1, wo_t[3], m,
                             first=False, close=True)
                    finish(psO_n3[mp][:, mh, :], m, 3)
            psO_o0 = []
            for m in range(4):
                pool, tag = ((psacc, "ps_a") if m < 3 else (psden, "ps_d"))
                psO = pool.tile([128, 512], F32, tag=tag, name="ps_o")
                half_mms(psO, att_od, 1, wo_t[0], m, first=True, close=True)
                finish(psO, m, 0, ev=ev0[m])
                psO_o0.append(psO)
            for mp in range(2):
                psO2 = psS.tile([128, 2, 512], F32, tag="ps_s", name="ps_o2")
                for mh in range(2):
                    m = 2 * mp + mh
                    half_mms(psO2[:, mh, :], att_od, 1, wo_t[1], m,
                             first=True, close=True)
                    finish(psO2[:, mh, :], m, 1, ev=ev1[m])

    nc.compile()
    return nc


def shard_inputs(cos, sin, hidden_states, w_qkv, w_o):
    """Host-side resharding into per-core input maps (data movement, layout
    transposes, and bf16 quantization of matmul operands)."""
    hs = np.asarray(hidden_states, dtype=np.float32)
    hT = np.ascontiguousarray(hs.reshape(TOK, HID).T.astype(BFNP))
    cosTt = np.ascontiguousarray(np.tile(np.asarray(cos, np.float32).T,
                                         (1, B)))
    sT = np.asarray(sin, np.float32).T                          # [128, 2048]
    sinST = np.concatenate([-sT[:64], sT[64:]], axis=0)
    sinST = np.ascontiguousarray(np.tile(sinST, (1, B)))        # [128, 4096]
    woT = np.ascontiguousarray(np.asarray(w_o, np.float32).T.astype(BFNP))
    ident = np.eye(128, dtype=np.float32)
    ones_col = np.ones((128, 1), BFNP)
    ones_row = np.ones((1, 128), BFNP)

    in_maps = []
    for c in range(NCORES):
        rows = [w_qkv[2 * c * D:(2 * c + 2) * D],
                w_qkv[(H + c) * D:(H + c + 1) * D],
                w_qkv[(H + HKV + c) * D:(H + HKV + c + 1) * D]]
        wq_c = np.concatenate(rows, axis=0).astype(np.float32)  # [512, 2048]
        wqT_c = np.ascontiguousarray(wq_c.T.astype(BFNP))       # [2048, 512]
        in_maps.append({
            "hT": hT, "wqT": wqT_c, "woT": woT,
            "cosT": cosTt, "sinST": sinST, "ident": ident,
            "ones_col": ones_col, "ones_row": ones_row,
        })
    return in_maps


_cached_nc = None


def kernel(cos, sin, hidden_states, w_qkv, w_o, _trace=False):
    global _cached_nc
    if _cached_nc is None:
        _cached_nc = build()
    nc = _cached_nc
    in_maps = shard_inputs(cos, sin, hidden_states, w_qkv, w_o)
    res = run_bass_kernel_spmd(nc, in_maps, core_ids=list(range(NCORES)),
                               trace=_trace)
    parts = [res.results[c]["out"] for c in range(NCORES)]
    full = np.concatenate(parts, axis=0).reshape(B, S, HID)
    out = np.ascontiguousarray(full.astype(np.float32))
    if _trace:
        return out, res
    return out


# revision 33
# speedup vs baseline: 1.1190x; 1.1190x over previous
"""GQA attention (B=2, S=2048, H=16, HKV=8, D=128) + RoPE + QKV/O proj
on 8 TRN2 NeuronCores.

Sharding: tensor-parallel by head. Core c computes QKV projection for its
2 q-heads / 1 kv-head over all tokens, RoPE, and full (non-causal)
attention for those heads.  An on-chip AllToAll then redistributes the
per-head attention outputs so core c holds *all* heads for its 512-token
block, and each core computes the output projection for its token block.
Host-side work is shard/concat plus bf16 quantization of the matmul
operands (the compute-precision choice for the PE; fp32 PSUM
accumulation throughout, fp32 softmax denominator).

All matmuls run with bf16 operands (TRN2 PE streams bf16 at 1
cycle/row vs 2 for fp32/f32r, and bf16 LDWEIGHTS is fast enough to
hide under the matmuls).  RoPE and softmax run in fp32 on the DVE/ACT
engines.  Softmax without max-subtraction (scores are in [-9, 9] for
this problem's data distribution).
"""
import sys
import types

import numpy as np
import ml_dtypes

BFNP = ml_dtypes.bfloat16


def _install_ntff_hook():
    """The container's antenv stub lacks axon_hooks; shim it so
    run_bass_kernel_spmd(trace=True) can capture NTFF profiles."""
    try:
        import antenv.axon_hooks  # noqa: F401
        return
    except ImportError:
        pass
    try:
        import trn_agent_boot.trn_boot as tb
        hook = tb._ntff_profile_via_ctypes("/opt/axon/libaxon_pjrt.so")
        mod = types.ModuleType("antenv.axon_hooks")
        mod.get_axon_ntff_profile_hook = lambda: hook
        sys.modules["antenv.axon_hooks"] = mod
    except Exception:
        pass


_install_ntff_hook()

import concourse.mybir as mybir  # noqa: E402
import concourse.tile as tile  # noqa: E402
from concourse import bacc  # noqa: E402
from concourse.bass_utils import run_bass_kernel_spmd  # noqa: E402

F32 = mybir.dt.float32
F32R = mybir.dt.float32r
BF16 = mybir.dt.bfloat16
AF = mybir.ActivationFunctionType

B, S, HID = 2, 2048, 2048
H, HKV, D = 16, 8, 128
NCORES = 8
TOK = B * S              # 4096 stacked tokens (batch-major)
TPC = TOK // NCORES      # 512 tokens owned per core
NTB = TOK // 512         # 8 phase-1 token blocks
KCH = HID // 128         # 16 contraction chunks
SCALE = 1.0 / float(np.sqrt(D))
HPC = H // NCORES        # 2 q-heads per core


def build():
    nc = bacc.Bacc("TRN2", target_bir_lowering=False, debug=False,
                   num_devices=NCORES)

    hT = nc.dram_tensor("hT", [HID, TOK], BF16, kind="ExternalInput")
    wqT = nc.dram_tensor("wqT", [HID, 4 * D], BF16, kind="ExternalInput")
    woT = nc.dram_tensor("woT", [H * D, HID], BF16, kind="ExternalInput")
    cosT = nc.dram_tensor("cosT", [D, TOK], F32, kind="ExternalInput")
    sinST = nc.dram_tensor("sinST", [D, TOK], F32, kind="ExternalInput")
    ones_col = nc.dram_tensor("ones_col", [128, 1], BF16, kind="ExternalInput")
    ones_row = nc.dram_tensor("ones_row", [1, 128], BF16, kind="ExternalInput")
    ident = nc.dram_tensor("ident", [128, 128], F32, kind="ExternalInput")
    out = nc.dram_tensor("out", [TPC, HID], F32, kind="ExternalOutput")

    hT_v = hT[:].rearrange("(c p) t -> p c t", p=128)      # [128, 16, 4096]
    wqT_v = wqT[:].rearrange("(c p) m -> p c m", p=128)    # [128, 16, 512]
    woT_v = woT[:].rearrange("(c p) o -> p c o", p=128)    # [128, 16, 2048]

    with tile.TileContext(nc) as tc:
        with (
            tc.tile_pool(name="cst", bufs=1) as cst,
            tc.tile_pool(name="res", bufs=1) as res,
            tc.tile_pool(name="med", bufs=16) as med,
            tc.tile_pool(name="tbl", bufs=2) as tblp,
            tc.tile_pool(name="wk", bufs=8) as wkp,
            tc.tile_pool(name="pp", bufs=3) as ppp,
            tc.tile_pool(name="rr", bufs=2) as rrp,
            tc.tile_pool(name="qs", bufs=3) as qsp,
            tc.tile_pool(name="aa", bufs=1) as aap,
            tc.tile_pool(name="wop", bufs=32) as wop,
            tc.tile_pool(name="psS", bufs=2, space="PSUM") as psS,
            tc.tile_pool(name="psacc", bufs=3, space="PSUM") as psacc,
            tc.tile_pool(name="psden", bufs=1, space="PSUM") as psden,
            tc.tile_pool(name="dram", bufs=1, space="DRAM") as dram,
        ):
            # ---- constants ----
            ident_s = cst.tile([128, 128], F32, name="ident_s")
            ident_b = cst.tile([128, 128], BF16, name="ident_b")
            ones_c = cst.tile([128, 1], BF16, name="ones_c")
            ones_r = cst.tile([1, 128], BF16, name="ones_r")
            nc.sync.dma_start(ident_s[:], ident[:])
            nc.sync.dma_start(ones_c[:], ones_col[:])
            nc.sync.dma_start(ones_r[:], ones_row[:])
            with nc.allow_low_precision(reason="bf16 identity"):
                nc.scalar.copy(ident_b[:], ident_s[:])

            # ---- resident tensors (bf16) ----
            wq_s = res.tile([128, KCH, 4 * D], BF16, name="wq_s")  # 16KB/p
            kT = res.tile([128, TOK], BF16, name="kT")             # 8KB/p
            V = res.tile([128, 32, 128], BF16, name="V")           # 8KB/p

            # DRAM bounces (bf16)
            qT_d = dram.tile([HPC, 128, TOK], BF16, name="qT_d")
            a2a_in = [
                dram.tile([NCORES, 128, TPC], BF16, name=f"a2a_in{h}")
                for h in range(HPC)
            ]
            a2a_out = [
                dram.tile([NCORES, 128, TPC], BF16, name=f"a2a_out{h}")
                for h in range(HPC)
            ]

            # ========== Phase 1 (QKV + RoPE) / Phase 2 (attention) ==========
            # Emitted batch-interleaved: tb 0-3 (batch 0) -> h0/b0 attention
            # -> tb 4-7 -> h0/b1 attention -> A2A0 -> h1 attention -> A2A1,
            # so the scheduler can fill phase-1 DMA stalls with attention
            # matmuls and vice versa.
            def ph1_block(tb, hook_a=None, hook_b=None):
                t0 = tb * 512
                hid_t = []
                if tb == 0:
                    # chunked loads on the (idle) scalar queue so the first
                    # matmuls only wait for their own contraction chunk
                    for hq in range(8):
                        nc.scalar.dma_start(wq_s[:, 2 * hq:2 * hq + 2, :],
                                            wqT_v[:, 2 * hq:2 * hq + 2, :])
                for hq in range(8):
                    ht_ = med.tile([128, 2, 512], BF16, tag="med",
                                   name=f"hid{hq}")
                    # tb1 splits across sync+scalar: both queues feed the
                    # HBM-bound startup window in parallel
                    eng = nc.scalar if (tb == 1 and hq % 2 == 1) else nc.sync
                    eng.dma_start(
                        ht_[:], hT_v[:, 2 * hq:2 * hq + 2, t0:t0 + 512])
                    hid_t.append(ht_)
                cosc = tblp.tile([128, 512], F32, tag="cosc", name="cosc")
                sinc = tblp.tile([128, 512], F32, tag="sinc", name="sinc")
                nc.sync.dma_start(cosc[:], cosT[:, t0:t0 + 512])
                nc.sync.dma_start(sinc[:], sinST[:, t0:t0 + 512])

                for m in range(4):  # q0, q1, k, v
                    ps = psacc.tile([128, 512], F32, tag="ps_a",
                                    name="ps_qkv")
                    for kk in range(KCH):
                        nc.tensor.matmul(
                            ps[:],
                            wq_s[:, kk, m * 128:(m + 1) * 128],
                            hid_t[kk // 2][:, kk % 2, :],
                            start=(kk == 0), stop=(kk == KCH - 1),
                        )
                    if m == 0 and hook_a is not None:
                        hook_a()
                    if m == 1 and hook_b is not None:
                        hook_b()
                    if m == 3:
                        # V: evict (on gpsimd - the ACT queue is exp-bound)
                        # then transpose to [t, d] layout
                        vt = wkp.tile([128, 512], BF16, tag="wkb", name="vt")
                        with nc.allow_low_precision(reason="bf16 V"):
                            nc.scalar.copy(vt[:], ps[:])
                        for i in range(4):
                            trp = psden.tile([128, 128], BF16, tag="ps_d",
                                             name="ps_tr")
                            nc.tensor.matmul(
                                trp[:], vt[:, i * 128:(i + 1) * 128],
                                ident_b[:], is_transpose=True,
                            )
                            with nc.allow_low_precision(reason="bf16 V"):
                                nc.scalar.copy(V[:, tb * 4 + i, :], trp[:])
                    else:
                        # q/k: RoPE.  dest = ps*cos + swap(ps)*sinS
                        qmul = wkp.tile([128, 512], F32, tag="wk",
                                        name="qmul")
                        qraw = wkp.tile([128, 512], F32, tag="wk", name="qraw")
                        rot = wkp.tile([128, 512], F32, tag="wk", name="rot")
                        nc.scalar.copy(qraw[:], ps[:])
                        nc.vector.tensor_mul(qmul[:], ps[:], cosc[:])
                        nc.gpsimd.dma_start(rot[0:64, :], qraw[64:128, :])
                        nc.gpsimd.dma_start(rot[64:128, :], qraw[0:64, :])
                        nc.vector.tensor_mul(rot[:], rot[:], sinc[:])
                        if m == 2:
                            dest = kT[:, t0:t0 + 512]
                        else:
                            dest = wkp.tile([128, 512], BF16, tag="wkb",
                                            name="qdest")
                        with nc.allow_low_precision(reason="bf16 q/k"):
                            nc.vector.tensor_add(dest[:], qmul[:], rot[:])
                        if m != 2:
                            # gpsimd queue: keeps this RoPE-gated write out
                            # of the sync queue's DMA-semaphore rotation,
                            # which would stall later hid loads behind it
                            nc.gpsimd.dma_start(qT_d[m, :, t0:t0 + 512],
                                                dest[:])

            # The softmax denominator chain (fold -> ones-matmul -> DVE
            # reciprocal -> rank-1 broadcast matmul) is a cross-engine
            # latency chain.  It is split off the main attention loop and
            # deferred into the *next* group's matmul stream so the two PE
            # matmuls embedded in it (psD, psB) never stall the strict-FIFO
            # PE queue: their inputs are ready by the time the PE drains
            # the next group's score/AV matmuls down to them.
            def ph2_main(h, b, qb, hook_a=None, hook_b=None):
                q0 = b * S + qb * 512
                qsl = qsp.tile([128, 512], BF16, tag="qs", name="qsl")
                nc.sync.dma_start(qsl[:], qT_d[h, :, q0:q0 + 512])
                psA = psacc.tile([128, 512], F32, tag="ps_a", name="ps_av")
                R = rrp.tile([128, 1024], F32, tag="rr", name="R")
                Pprev = None
                for i in range(8):  # pairs of 128-token k/v blocks
                    ta = b * S + (2 * i) * 128
                    tbk = b * S + (2 * i + 1) * 128
                    psSt = psS.tile([128, 1024], F32, tag="ps_s",
                                    name="ps_sc")
                    nc.tensor.matmul(psSt[:, 0:512],
                                     kT[:, ta:ta + 128], qsl[:],
                                     start=True, stop=True)
                    nc.tensor.matmul(psSt[:, 512:1024],
                                     kT[:, tbk:tbk + 128], qsl[:],
                                     start=True, stop=True)
                    P = ppp.tile([128, 1024], BF16, tag="pp", name="P")
                    with nc.allow_low_precision(reason="bf16 attn probs"):
                        nc.scalar.activation(P[:], psSt[:], AF.Exp,
                                             scale=SCALE)
                    nc.tensor.matmul(psA[:], V[:, b * 16 + 2 * i, :],
                                     P[:, 0:512],
                                     start=(i == 0), stop=False)
                    nc.tensor.matmul(psA[:], V[:, b * 16 + 2 * i + 1, :],
                                     P[:, 512:1024],
                                     start=False, stop=(i == 7))
                    if i == 1:
                        nc.vector.tensor_add(R[:], Pprev[:], P[:])
                    elif i >= 2:
                        nc.vector.tensor_add(R[:], R[:], P[:])
                    Pprev = P
                    if i == 2 and hook_a is not None:
                        hook_a()
                    if i == 5 and hook_b is not None:
                        hook_b()
                return {"h": h, "b": b, "qb": qb, "psA": psA, "R": R}

            def ph2_tail_a(ctx):
                R = ctx["R"]
                R2 = wkp.tile([128, 512], BF16, tag="wkb", name="R2")
                with nc.allow_low_precision(reason="bf16 softmax denom"):
                    nc.vector.tensor_add(R2[:], R[:, 0:512], R[:, 512:1024])
                psD = psden.tile([1, 512], F32, tag="ps_d", name="ps_den")
                nc.tensor.matmul(psD[:], ones_c[:], R2[:],
                                 start=True, stop=True)
                rec32 = wkp.tile([1, 512], F32, tag="wk", name="rec32")
                nc.vector.reciprocal_approx_fast(rec32[:], psD[:])
                rec = wkp.tile([1, 512], BF16, tag="wkb", name="rec")
                with nc.allow_low_precision(
                        reason="bf16 rounding of softmax denom"):
                    nc.vector.tensor_copy(rec[:], rec32[:])
                ctx["rec"] = rec

            def ph2_tail_b(ctx):
                psB = psden.tile([128, 512], F32, tag="ps_d", name="ps_bc")
                nc.tensor.matmul(psB[:], ones_r[:], ctx["rec"][:],
                                 start=True, stop=True)
                bcB = wkp.tile([128, 512], F32, tag="wk", name="bcB")
                nc.scalar.copy(bcB[:], psB[:])
                attn = wkp.tile([128, 512], BF16, tag="wkb", name="attn")
                with nc.allow_low_precision(reason="bf16 attention values"):
                    nc.vector.tensor_mul(attn[:], ctx["psA"][:], bcB[:])
                nc.sync.dma_start(a2a_in[ctx["h"]][4 * ctx["b"] + ctx["qb"]],
                                  attn[:])

            pending = [None]

            def _hook_a():
                if pending[0] is not None:
                    ph2_tail_a(pending[0])

            def _hook_b():
                if pending[0] is not None:
                    ph2_tail_b(pending[0])
                    pending[0] = None

            def ph2_group(h, b, qb):
                ctx = ph2_main(h, b, qb, hook_a=_hook_a, hook_b=_hook_b)
                pending[0] = ctx

            def ph2_flush():
                if pending[0] is not None:
                    ph2_tail_a(pending[0])
                    ph2_tail_b(pending[0])
                    pending[0] = None

            # ================= Phase 3: output projection =================
            # wo tiles live in their own pool (no ring-slot dependency on
            # the hid tiles) and load via the gpsimd queue so they are not
            # stuck behind phase-2 traffic on the sync engine's DMA queue.
            def wo_load(n):
                n0 = n * 512
                tiles = []
                for wq4 in range(8):
                    wt_ = wop.tile([128, 2, 512], BF16, tag="wop",
                                   name=f"wo{wq4}")
                    nc.gpsimd.dma_start(
                        wt_[:], woT_v[:, 2 * wq4:2 * wq4 + 2, n0:n0 + 512])
                    tiles.append(wt_)
                return tiles

            def half_mms(psO, att, kh0, wo_t, m, first, close):
                for j in range(8):
                    kh = 2 * j + kh0
                    nc.tensor.matmul(
                        psO[:],
                        att[:, j, m * 128:(m + 1) * 128],
                        wo_t[kh // 2][:, kh % 2, :],
                        start=(first and j == 0), stop=(close and j == 7),
                    )

            def finish(psO, m, n, ev=None):
                oev = wkp.tile([128, 512], F32, tag="wk", name="oev")
                if ev is None:
                    nc.scalar.copy(oev[:], psO[:])
                else:
                    nc.vector.tensor_add(oev[:], psO[:], ev[:])
                nc.sync.dma_start(out[m * 128:(m + 1) * 128,
                                      n * 512:(n + 1) * 512], oev[:])

            # ---------------- emission schedule ----------------
            # tb 4-7 are woven between the first attention groups: the PE
            # queue is strict FIFO, so a ph2 group whose inputs are still
            # settling must have ready QKV matmuls emitted *before* it,
            # not after.
            for tb in range(4):
                ph1_block(tb)
            ph1_block(4)
            ph2_group(0, 0, 0)
            ph1_block(5, _hook_a, _hook_b)
            ph2_group(0, 0, 1)
            ph1_block(6, _hook_a, _hook_b)
            ph2_group(0, 0, 2)
            ph1_block(7, _hook_a, _hook_b)
            ph2_group(0, 0, 3)
            for qb in range(4):
                ph2_group(0, 1, qb)
            ph2_flush()
            nc.gpsimd.collective_compute(
                "AllToAll", mybir.AluOpType.bypass,
                replica_groups=[list(range(NCORES))],
                ins=[a2a_in[0].opt()], outs=[a2a_out[0].opt()],
            )
            # o-proj weight prefetch sits on the gpsimd queue behind the
            # A2A0 trigger: no HBM contention with phase-1 loads, done
            # well before phase 3 needs them.
            wo_t = [wo_load(n) for n in range(4)]
            att_ev = aap.tile([128, 8, 512], BF16, name="att_ev")
            for j in range(NCORES):
                nc.gpsimd.dma_start(att_ev[:, j, :], a2a_out[0][j])
            for b in range(B):
                for qb in range(4):
                    ph2_group(1, b, qb)
            ph2_flush()
            nc.gpsimd.collective_compute(
                "AllToAll", mybir.AluOpType.bypass,
                replica_groups=[list(range(NCORES))],
                ins=[a2a_in[1].opt()], outs=[a2a_out[1].opt()],
            )
            # While the AllToAll is in flight the PE runs all the even-head
            # (att_ev) halves: n=0,1 as closed partial groups that are
            # evicted to SBUF (bf16), freeing the banks for n=2,3 whose
            # groups stay open across the collective.
            def even_quad(wo_n, close):
                tiles = []
                for m in range(4):
                    pool, tag = ((psacc, "ps_a") if m < 3 else
                                 (psden, "ps_d"))
                    psO = pool.tile([128, 512], F32, tag=tag, name="ps_o")
                    half_mms(psO, att_ev, 0, wo_n, m, first=True,
                             close=close)
                    tiles.append(psO)
                return tiles

            def even_pair(wo_n, close):
                tiles = []
                for mp in range(2):
                    psO2 = psS.tile([128, 2, 512], F32, tag="ps_s",
                                    name="ps_o2")
                    for mh in range(2):
                        half_mms(psO2[:, mh, :], att_ev, 0, wo_n,
                                 2 * mp + mh, first=True, close=close)
                    tiles.append(psO2)
                return tiles

            psO_n0 = even_quad(wo_t[0], close=True)
            ev0 = []
            for m in range(4):
                ev = wkp.tile([128, 512], BF16, tag="ev", name="ev0")
                with nc.allow_low_precision(reason="bf16 o-proj partial"):
                    nc.scalar.copy(ev[:], psO_n0[m][:])
                ev0.append(ev)
            psO_n1 = even_pair(wo_t[1], close=True)
            ev1 = []
            for mp in range(2):
                for mh in range(2):
                    ev = wkp.tile([128, 512], BF16, tag="ev", name="ev1")
                    with nc.allow_low_precision(reason="bf16 o-proj partial"):
                        nc.scalar.copy(ev[:], psO_n1[mp][:, mh, :])
                    ev1.append(ev)
            psO_n2 = even_quad(wo_t[2], close=False)
            psO_n3 = even_pair(wo_t[3], close=False)
            att_od = aap.tile([128, 8, 512], BF16, name="att_od")
            for j in range(NCORES):
                nc.sync.dma_start(att_od[:, j, :], a2a_out[1][j])
            # odd-head halves: n=2,3 accumulate into the open groups;
            # n=0,1 restart in the freed banks and add back the evicted
            # even partials during the final copy.
            for m in range(4):
                half_mms(psO_n2[m], att_od, 1, wo_t[2], m, first=False,
                         close=True)
                finish(psO_n2[m], m, 2)
            for mp in range(2):
                for mh in range(2):
                    m = 2 * mp + mh
                    half_mms(psO_n3[mp][:, mh, :], att_od, 1, wo_t[3], m,
                             first=False, close=True)
                    finish(psO_n3[mp][:, mh, :], m, 3)
            psO_o0 = []
            for m in range(4):
                pool, tag = ((psacc, "ps_a") if m < 3 else (psden, "ps_d"))
                psO = pool.tile([128, 512], F32, tag=tag, name="ps_o")
                half_mms(psO, att_od, 1, wo_t[0], m, first=True, close=True)
                finish(psO, m, 0, ev=ev0[m])
                psO_o0.append(psO)
            for mp in range(2):
                psO2 = psS.tile([128, 2, 512], F32, tag="ps_s", name="ps_o2")
                for mh in range(2):
                    m = 2 * mp + mh
                    half_mms(psO2[:, mh, :], att_od, 1, wo_t[1], m,
                             first=True, close=True)
                    finish(psO2[:, mh, :], m, 1, ev=ev1[m])

    nc.compile()
    return nc


def shard_inputs(cos, sin, hidden_states, w_qkv, w_o):
    """Host-side resharding into per-core input maps (data movement, layout
    transposes, and bf16 quantization of matmul operands)."""
    hs = np.asarray(hidden_states, dtype=np.float32)
    hT = np.ascontiguousarray(hs.reshape(TOK, HID).T.astype(BFNP))
    cosTt = np.ascontiguousarray(np.tile(np.asarray(cos, np.float32).T,
                                         (1, B)))
    sT = np.asarray(sin, np.float32).T                          # [128, 2048]
    sinST = np.concatenate([-sT[:64], sT[64:]], axis=0)
    sinST = np.ascontiguousarray(np.tile(sinST, (1, B)))        # [128, 4096]
    woT = np.ascontiguousarray(np.asarray(w_o, np.float32).T.astype(BFNP))
    ident = np.eye(128, dtype=np.float32)
    ones_col = np.ones((128, 1), BFNP)
    ones_row = np.ones((1, 128), BFNP)

    in_maps = []
    for c in range(NCORES):
        rows = [w_qkv[2 * c * D:(2 * c + 2) * D],
                w_qkv[(H + c) * D:(H + c + 1) * D],
                w_qkv[(H + HKV + c) * D:(H + HKV + c + 1) * D]]
        wq_c = np.concatenate(rows, axis=0).astype(np.float32)  # [512, 2048]
        wqT_c = np.ascontiguousarray(wq_c.T.astype(BFNP))       # [2048, 512]
        in_maps.append({
            "hT": hT, "wqT": wqT_c, "woT": woT,
            "cosT": cosTt, "sinST": sinST, "ident": ident,
            "ones_col": ones_col, "ones_row": ones_row,
        })
    return in_maps


_cached_nc = None


def kernel(cos, sin, hidden_states, w_qkv, w_o, _trace=False):
    global _cached_nc
    if _cached_nc is None:
        _cached_nc = build()
    nc = _cached_nc
    in_maps = shard_inputs(cos, sin, hidden_states, w_qkv, w_o)
    res = run_bass_kernel_spmd(nc, in_maps, core_ids=list(range(NCORES)),
                               trace=_trace)
    parts = [res.results[c]["out"] for c in range(NCORES)]
    full = np.concatenate(parts, axis=0).reshape(B, S, HID)
    out = np.ascontiguousarray(full.astype(np.float32))
    if _trace:
        return out, res
    return out
